# revision 46
# baseline (speedup 1.0000x reference)
"""Bidirectional-GRU encoding layer for Trainium2 (8 NeuronCores, Bass/Tile).

The reference computes a length-masked bidirectional GRU over [B=32, T=512,
D=512] and returns gru_outputs[:, -1, :] (shape [B, 2H]).  dynamic_rnn
masking means output rows are exactly zero for every sample with
length < T, and for samples with length == T the row is
    [ fw_h_after_T_steps , (1-u)*c of a single bw GRU step on x[T-1] ].
The kernel computes exactly that: a single-step bw-GRU candidate for all
samples (masked by length==T) always runs on-device; the 512-step fw scan
is only compiled/run when at least one sample has length == T.

Sharding: data-parallel over batch, 4 samples per core (weights replicated).
Compute layout is feature-on-partition (everything transposed), so the
sequential scan's elementwise chain runs on [128, few] tiles.  Matmul
operands (weights, x, h state) are fp16 with fp32 PSUM accumulation and
fp32 gate math — fp32 matmuls cost two PE passes on trn2 and the scan is
weight-load-bound; fp16 keeps the end-to-end error ~6e-4.  The u-gate
weight columns are pre-negated on the host so sigmoid yields v = 1-u
directly, shortening the post-tanh critical path of each scan step.

The scan step's critical path is engine-hop bound (PE -> ACT -> DVE ->
PE -> ACT -> DVE), and was tightened from ~3.6us to ~2.6us/step by:
 - injecting the per-step XG bias into PSUM with identity matmuls
   (one per bank; start=True clears the whole bank's has_written bits)
   that run in PE idle windows, so sigmoid/tanh read PSUM directly and
   three DVE adds leave the path;
 - distributing the next step's r-gate contraction over
   h' = ah + bt (ah = u*h, bt = (1-u)*c): the ah-part matmuls pre-run
   during the tanh window and only bt gates the final 16 r-matmuls, so
   the h-materialization add leaves the path.  The v-matmuls run early
   in the body (they need only the old h) so sigmoid_v -> ah completes
   before the ah-part needs it;
 - a PE warm-up burst at kernel start (HAM clock gate at 2.4 GHz for
   phase A/B), contiguous phase-B activations (t innermost in XG),
   input DMAs spread over the sync/scalar/gpsimd queues, and a
   256-step unroll (hw-loop boundaries drain all engines, ~5us each).
"""

import numpy as np

B, T, D, H = 32, 512, 512, 512
N_CORES = 8
BPC = B // N_CORES  # 4 samples per core
P = 128
KD = D // P  # 4 k-tiles over the depth dim
MH = H // P  # 4 m-tiles over the hidden dim
NG = (2 * H + H) // P  # 12 m-tiles over [ru | c] gate outputs

SCAN_UNROLL = 512
_CACHE = {}
TRACE = False          # test harness sets True to capture an NTFF profile
LAST_RESULT = None     # BassKernelResults of the most recent run


def _bf16():
    return np.float16


def _build_kernel(with_scan: bool):
    import concourse.mybir as mybir
    import concourse.tile as tile
    from concourse import bacc
    from concourse.bass import ds, ts

    f32 = mybir.dt.float32
    bf16 = mybir.dt.float16
    wdt = mybir.dt.float16
    AF = mybir.ActivationFunctionType

    nc = bacc.Bacc("TRN2", target_bir_lowering=False, debug=False,
                   num_devices=N_CORES)

    # --- DRAM I/O (per-core shards) ---
    # wA = [ -bw_gk_u | bw_ck | xlastT ] columns; sA = [ -bu | bc | mask ]
    # (single fp16 + single f32 input DMA for the bw phase)
    wA_d = nc.dram_tensor("wA", [P, KD, H + BPC], wdt,
                          kind="ExternalInput").ap()
    wB_d = nc.dram_tensor("wB", [P, KD, H], wdt, kind="ExternalInput").ap()
    sA_d = nc.dram_tensor("sA", [P, 3 * MH, BPC], f32, kind="ExternalInput").ap()
    if with_scan:
        fwWx_d = nc.dram_tensor("fwWx", [D, 3 * H], bf16, kind="ExternalInput").ap()
        fwWh_d = nc.dram_tensor("fwWh", [H, 3 * H], bf16, kind="ExternalInput").ap()
        fwb_d = nc.dram_tensor("fwb", [P, NG], f32, kind="ExternalInput").ap()
        ident_d = nc.dram_tensor("ident", [P, P], bf16, kind="ExternalInput").ap()
        xscanT_d = nc.dram_tensor("xscanT", [BPC, D, T], bf16,
                                  kind="ExternalInput").ap()
    outT_d = nc.dram_tensor("outT", [2 * H, BPC], f32, kind="ExternalOutput").ap()
    # view as [P, 8, BPC]: row (a*128+p) -> [p, a, s]; a=0..3 fw, a=4..7 bw
    out_v = outT_d.rearrange("(a p) s -> p a s", p=P)

    with tile.TileContext(nc) as tc:
        with (
            tc.tile_pool(name="const", bufs=1) as cpool,
            tc.tile_pool(name="work", bufs=4) as wpool,
        ):
            # ---------- Phase A: single-step bw candidate, masked ----------
            # warm the ACT function table during the DMA phase
            warm = wpool.tile([P, 1], f32, tag="warm")
            nc.vector.memset(warm[:], 0.0)
            warm2 = wpool.tile([P, 1], f32, tag="warm2")
            nc.scalar.activation(warm2[:], warm[:], AF.Sigmoid)
            # Warm the PE HAM clock gate during the DMA lead-in: ~8us of
            # sustained dummy matmuls flips the PE from 1.2 to 2.4 GHz, so
            # the phase A/B matmuls (and the scan) start at the warm clock
            # instead of paying the cold rate for their whole duration.
            if with_scan:
                wsrc = wpool.tile([P, 5 * P], wdt, tag="wsrc")
                nc.vector.memset(wsrc[:], 0.0)
                with tc.tile_pool(name="psumW", bufs=2, space="PSUM") as ppoolW:
                    for i in range(20):
                        pw = ppoolW.tile([P, 4 * P], f32, tag="pw")
                        nc.tensor.matmul(pw[:], wsrc[:, 0:P],
                                         wsrc[:, P:5 * P], start=True,
                                         stop=True)

            # xs[0] gates the phase-B matmul chain, so its transfer goes
            # first on the sync queue; phase A (wA/wB) has slack.
            xs0 = None
            if with_scan:
                xs0 = cpool.tile([P, KD, T], bf16, tag="xs0")
                nc.sync.dma_start(
                    xs0[:], xscanT_d[0].rearrange("(k p) t -> p k t", p=P))
            # wA is pre-transposed to [P, KD, M] on the host so each
            # partition's DMA read is contiguous; the load is split across
            # the two HWDGE queues (sync + scalar) to overlap transfers
            wA = cpool.tile([P, KD, H + BPC], wdt, tag="wA")
            nc.sync.dma_start(wA[:], wA_d[:])
            wB = cpool.tile([P, KD, H], wdt, tag="wB")
            nc.scalar.dma_start(wB[:], wB_d[:])
            sA = cpool.tile([P, 3 * MH, BPC], f32, tag="sA")
            nc.scalar.dma_start(sA[:], sA_d[:])
            xlast = wA[:, :, H:H + BPC]
            maskv = sA[:, 2 * MH:3 * MH, :]

            # out_sb holds the full transposed output row block for this core
            out_sb = cpool.tile([P, 2 * MH, BPC], f32, tag="out_sb")
            nc.vector.memset(out_sb[:], 0.0)

            with tc.tile_pool(name="psumA", bufs=1, space="PSUM") as ppoolA:
                pz = ppoolA.tile([P, 2 * MH, BPC], f32, tag="pz")
                for m in range(2 * MH):
                    w = wA if m < MH else wB
                    mm = m if m < MH else m - MH
                    for k in range(KD):
                        nc.tensor.matmul(pz[:, m, :], w[:, k, ts(mm, P)],
                                         xlast[:, k, :], start=(k == 0),
                                         stop=(k == KD - 1))
                z = wpool.tile([P, 2 * MH, BPC], f32, tag="z")
                nc.vector.tensor_add(z[:], pz[:], sA[:, 0:2 * MH, :])
                u1 = wpool.tile([P, MH, BPC], f32, tag="u1")   # 1-u = sigmoid(-z)
                nc.scalar.activation(u1[:], z[:, 0:MH, :], AF.Sigmoid)
                cc = wpool.tile([P, MH, BPC], f32, tag="cc")
                nc.scalar.activation(cc[:], z[:, MH:2 * MH, :], AF.Tanh)
                bwcand = wpool.tile([P, MH, BPC], f32, tag="bwcand")
                nc.vector.tensor_mul(bwcand[:], u1[:], cc[:])
                nc.vector.tensor_mul(out_sb[:, MH:2 * MH, :], bwcand[:], maskv[:])

            if not with_scan:
                # fw half stays exactly zero (no length==T sample)
                nc.sync.dma_start(out_v[:], out_sb[:])

            # ---------- Phase B: x-projections for all t (if scanning) -----
            if with_scan:
                fwb = cpool.tile([P, NG], f32, tag="fwb")
                nc.gpsimd.dma_start(fwb[:], fwb_d[:])
                ident = cpool.tile([P, P], bf16, tag="ident")
                nc.gpsimd.dma_start(ident[:], ident_d[:])

                # XG[p, m, s, t] = (x_s[t] @ fwWx + fwb)[m*128+p]; fp16 so it
                # can be the moving operand of the PSUM bias-inject matmuls.
                # t is the innermost dim so the phase-B bias-activations write
                # contiguously (strided ACT writes measured ~4x slower), padded
                # by one step for the scan's xg prefetch of t+1.
                XG = cpool.tile([P, NG, BPC, T + 2], bf16, tag="XG")
                with (
                    tc.tile_pool(name="xpre", bufs=3) as xpool,
                    tc.tile_pool(name="psumB", bufs=8, space="PSUM") as ppoolB,
                ):
                    fwWx = xpool.tile([P, KD, 3 * H], bf16, tag="fwWx")
                    fwWx_v = fwWx_d.rearrange("(k p) m -> p k m", p=P)
                    nc.gpsimd.dma_start(fwWx[:, :, 0:H], fwWx_v[:, :, 0:H])
                    nc.gpsimd.dma_start(fwWx[:, :, H:3 * H],
                                        fwWx_v[:, :, H:3 * H])
                    # fwWh is only needed once the scan starts ~45us later;
                    # queue it behind fwWx so it never delays phase B
                    fwWh = cpool.tile([P, KD, 3 * H], bf16, tag="fwWh")
                    nc.gpsimd.dma_start(fwWh[:],
                                        fwWh_d.rearrange("(k p) m -> p k m", p=P))
                    # the sample inputs alternate between two queues so the
                    # phase-B matmuls of sample s are never gated by the
                    # transfers of s-1 (xs[0] was prefetched above)
                    xq = [nc.sync, nc.scalar, nc.sync, nc.scalar]
                    for s in range(BPC):
                        if s == 0:
                            xs = xs0
                        else:
                            xs = xpool.tile([P, KD, T], bf16, tag="xs")
                            xq[s].dma_start(
                                xs[:],
                                xscanT_d[s].rearrange("(k p) t -> p k t", p=P))
                        for m in range(NG):
                            pxg = ppoolB.tile([P, T], f32, tag="pxg")
                            for k in range(KD):
                                nc.tensor.matmul(pxg[:], fwWx[:, k, ts(m, P)],
                                                 xs[:, k, :], start=(k == 0),
                                                 stop=(k == KD - 1))
                            nc.scalar.activation(XG[:, m, s, 0:T], pxg[:],
                                                 AF.Identity, bias=fwb[:, m:m + 1])

                # ---------- Phase C: the sequential scan -------------------
                # state lives in fp16 (matmul operand dtype) the whole time
                hT = cpool.tile([P, 1, MH, BPC], bf16, tag="hT")
                nc.vector.memset(hT[:], 0.0)

                # The ident matmuls read XG[..., t] directly with the loop
                # register offset on the moving operand: the dynamic AP costs
                # ~170ns of PE issue per matmul (NX address ALU), but all
                # three run in the PE idle window during the previous step's
                # tanh, and dropping the DVE staging copies keeps the
                # critical bt->hT chain tight.
                # Software-pipelined across the step boundary: body t
                # consumes the r/c gate psums emitted in body t-1, and the
                # r-gate contraction for t+1 is DISTRIBUTED over
                # h(t) = ah(t) + bt(t)  (ah = u*h, bt = (1-u)*c): the
                # ah-part matmuls pre-run during the tanh window, so after
                # tanh only the bt multiply gates the last 16 r-matmuls and
                # the h-materialization add leaves the critical path.  The
                # v-matmuls run early in the body (they need only the old h)
                # so sigmoid_v -> ah completes before the ah-part needs it.
                with (
                    tc.tile_pool(name="psumR", bufs=4, space="PSUM") as ppoolR,
                    tc.tile_pool(name="psumC", bufs=2, space="PSUM") as ppoolC,
                ):
                    pg_r_c = ppoolR.tile([P, 1, MH, BPC], f32, tag="pg_r")
                    # bootstrap: h(-1)=0, so r-gates(0) psum = XG bias only
                    # (ONE ident matmul per bank: start=True clears the whole
                    # bank's has_written bits)
                    nc.tensor.matmul(pg_r_c[:, 0, :, :], ident[:],
                                     XG[:, 0:MH, :, 0:1],
                                     start=True, stop=True)

                    def step(t):
                        nonlocal pg_r_c
                        g_r = wpool.tile([P, 1, MH, BPC], bf16, tag="g_r")
                        nc.scalar.activation(g_r[:], pg_r_c[:], AF.Sigmoid)
                        rh = wpool.tile([P, 1, MH, BPC], bf16, tag="rh")
                        nc.vector.tensor_mul(rh[:], g_r[:], hT[:])

                        # v gates of step t (need only h(t-1)): run during
                        # the sigmoid_r window so sigmoid_v / ah are ready
                        # before the ah-part matmuls below
                        pg_v = ppoolC.tile([P, 1, MH, BPC], f32, tag="pg_v")
                        nc.tensor.matmul(pg_v[:, 0, :, :], ident[:],
                                         XG[:, MH:2 * MH, :, ds(t, 1)],
                                         start=True, stop=False)
                        for m in range(MH):
                            for k in range(KD):
                                nc.tensor.matmul(pg_v[:, 0, m, :],
                                                 fwWh[:, k, ts(MH + m, P)],
                                                 hT[:, 0, k, :], start=False,
                                                 stop=(k == KD - 1))

                        pcs = ppoolC.tile([P, 1, MH, BPC], f32, tag="pcs")
                        nc.tensor.matmul(pcs[:, 0, :, :], ident[:],
                                         XG[:, 2 * MH:NG, :, ds(t, 1)],
                                         start=True, stop=False)
                        for m in range(MH):
                            for k in range(KD):
                                nc.tensor.matmul(pcs[:, 0, m, :],
                                                 fwWh[:, k, ds(2 * H + m * P, P)],
                                                 rh[:, 0, k, :], start=False,
                                                 stop=(k == KD - 1))
                        g_v = wpool.tile([P, 1, MH, BPC], f32, tag="g_v")
                        nc.scalar.activation(g_v[:], pg_v[:], AF.Sigmoid)
                        a2 = wpool.tile([P, 1, MH, BPC], f32, tag="a2")
                        nc.vector.tensor_mul(a2[:], g_v[:], hT[:])
                        ah = wpool.tile([P, 1, MH, BPC], bf16, tag="ah")
                        nc.vector.tensor_sub(ah[:], hT[:], a2[:])

                        # next-step r-gate group: bias + ah-part (preruns
                        # in the tanh window); fully unrolled, so the last
                        # step skips the unused t+1 group
                        last = (t + 1 >= T)
                        if not last:
                            pg_r_n = ppoolR.tile([P, 1, MH, BPC], f32,
                                                 tag="pg_r")
                            nc.tensor.matmul(pg_r_n[:, 0, :, :], ident[:],
                                             XG[:, 0:MH, :, ds(t + 1, 1)],
                                             start=True, stop=False)
                            for m in range(MH):
                                for k in range(KD):
                                    nc.tensor.matmul(pg_r_n[:, 0, m, :],
                                                     fwWh[:, k, ts(m, P)],
                                                     ah[:, 0, k, :],
                                                     start=False, stop=False)

                        ct = wpool.tile([P, 1, MH, BPC], f32, tag="ct")
                        nc.scalar.activation(ct[:], pcs[:], AF.Tanh)
                        bt = wpool.tile([P, 1, MH, BPC], bf16, tag="bt")
                        nc.vector.tensor_mul(bt[:], g_v[:], ct[:])

                        # bt-part closes the next r-gate group
                        if not last:
                            for m in range(MH):
                                for k in range(KD):
                                    nc.tensor.matmul(pg_r_n[:, 0, m, :],
                                                     fwWh[:, k, ts(m, P)],
                                                     bt[:, 0, k, :],
                                                     start=False,
                                                     stop=(m == MH - 1 and
                                                           k == KD - 1))
                        # h' = u*h + (1-u)*c, fp16 state; off the critical
                        # path (v/c groups and next elementwise consume it)
                        nc.vector.tensor_add(hT[:], ah[:], bt[:])

                        if not last:
                            pg_r_c = pg_r_n

                    if SCAN_UNROLL > 1:
                        def ubody(iv0, unroll):
                            for i in range(unroll):
                                step(iv0 + i)
                        tc.For_i_unrolled_general(
                            0, T, 1, ubody, max_unroll=SCAN_UNROLL,
                            hint_engines=(mybir.EngineType.PE,
                                          mybir.EngineType.DVE))
                    else:
                        with tc.For_i(0, T, 1) as t:
                            step(t)

                nc.vector.tensor_mul(out_sb[:, 0:MH, :], hT[:, 0, :, :], maskv[:])
                nc.sync.dma_start(out_v[:], out_sb[:])

    nc.compile()
    return nc


def _get_kernel(with_scan: bool):
    key = ("scan" if with_scan else "noscan")
    if key not in _CACHE:
        _CACHE[key] = _build_kernel(with_scan)
    return _CACHE[key]


def host_inputs(inputs, fw_gk, fw_gb, fw_ck, fw_cb,
                bw_gk, bw_gb, bw_ck, bw_cb, length):
    """Shard/transpose/cast the full inputs into per-core in_maps."""
    bf16 = _bf16()
    inputs = np.asarray(inputs, dtype=np.float32)
    length = np.asarray(length)
    mask = (length.astype(np.int64) >= T).astype(np.float32)  # [B]
    with_scan = bool(mask.any())

    fw_gk = np.asarray(fw_gk, np.float32)
    fw_ck = np.asarray(fw_ck, np.float32)
    bw_gk = np.asarray(bw_gk, np.float32)
    bw_ck = np.asarray(bw_ck, np.float32)
    fw_gb = np.asarray(fw_gb, np.float32)
    fw_cb = np.asarray(fw_cb, np.float32)
    bw_gb = np.asarray(bw_gb, np.float32)
    bw_cb = np.asarray(bw_cb, np.float32)

    wdt = bf16
    bwW = np.concatenate([-bw_gk[:D, H:2 * H], bw_ck[:D]], axis=1).astype(wdt)
    # per-partition biases laid out [P, m-tile], broadcast over samples
    bias_uc = np.concatenate([-bw_gb[H:2 * H], bw_cb]).reshape(2 * MH, P).T
    bias_bc = np.broadcast_to(bias_uc[:, :, None], (P, 2 * MH, BPC))
    shared = {}
    if with_scan:
        # u-gate columns pre-negated: sigmoid then yields v = 1-u directly
        neg = np.ones((1, 3 * H), np.float32)
        neg[:, H:2 * H] = -1.0
        shared["fwWx"] = np.ascontiguousarray(
            (np.concatenate([fw_gk[:D], fw_ck[:D]], axis=1) * neg).astype(bf16))
        shared["fwWh"] = np.ascontiguousarray(
            (np.concatenate([fw_gk[D:], fw_ck[D:]], axis=1) * neg).astype(bf16))
        fwb_full = np.concatenate([fw_gb, fw_cb]) * neg[0]
        shared["fwb"] = np.ascontiguousarray(fwb_full.reshape(NG, P).T)
        shared["ident"] = np.eye(P, dtype=bf16)

    in_maps = []
    for c in range(N_CORES):
        sl = slice(c * BPC, (c + 1) * BPC)
        m = dict(shared)
        wa2 = np.concatenate([bwW[:, 0:H], inputs[sl, T - 1, :].T.astype(wdt)],
                             axis=1)
        m["wA"] = np.ascontiguousarray(
            wa2.reshape(KD, P, H + BPC).transpose(1, 0, 2))
        m["wB"] = np.ascontiguousarray(
            bwW[:, H:2 * H].reshape(KD, P, H).transpose(1, 0, 2))
        mask_bc = np.broadcast_to(mask[sl][None, None, :], (P, MH, BPC))
        m["sA"] = np.ascontiguousarray(
            np.concatenate([bias_bc, mask_bc], axis=1), dtype=np.float32)
        if with_scan:
            m["xscanT"] = np.ascontiguousarray(
                inputs[sl].transpose(0, 2, 1).astype(bf16))
        in_maps.append(m)
    return with_scan, in_maps


def kernel(inputs, fw_gk, fw_gb, fw_ck, fw_cb,
           bw_gk, bw_gb, bw_ck, bw_cb, length):
    from concourse.bass_utils import run_bass_kernel_spmd

    with_scan, in_maps = host_inputs(inputs, fw_gk, fw_gb, fw_ck, fw_cb,
                                     bw_gk, bw_gb, bw_ck, bw_cb, length)
    nc = _get_kernel(with_scan)
    res = run_bass_kernel_spmd(nc, in_maps, core_ids=list(range(N_CORES)),
                               trace=TRACE)
    global LAST_RESULT
    LAST_RESULT = res

    out = np.empty((B, 2 * H), np.float32)
    for c in range(N_CORES):
        out[c * BPC:(c + 1) * BPC] = res.results[c]["outT"].T
    return out

